# revision 6
# baseline (speedup 1.0000x reference)
"""Trainium2 Bass kernel for nn_DecodeSBP (keypoint heatmap decode).

Contract: kernel(x=[1,133,512,512] f32) -> [133,3] f32
  joints[k] = (4*xx, 4*yy, conf) if conf > 0.8 else (-4, -4, -1)
  where flat = argmax(sigmoid(x[0,k])), conf = sigmoid(max), yy = flat//512,
  xx = flat%512. sigmoid is monotonic so the argmax runs on raw logits.

Sharding: keypoint dim across 8 cores (17/core, core 7 zero-padded).

Per-core program (one full-data pass, hierarchical argmax, chunk-split
select):
  stream: one 1 MB DMA per keypoint for k0..k14; k15 and k16 stream in
    quarter DMAs so their reductions trail the stream tail closely.
    DVE reduce_max emits per-(partition, 512-chunk) maxes right behind
    each tile's completion semaphore.
  select (pipelined per chunk column): as soon as chunk c's maxes are
    complete for all 17 keypoints (i.e. right after k16's quarter-c
    reduce), TensorE transposes them into PSUM [17, 128] and DVE runs a
    per-chunk max8/max_index + rank decode (chunk-c column j is
    partition j, so rank = 4*j + c needs ONE fused op). Only chunk 3's
    select + a tiny 32-wide merge remain after the last streamed byte.
  merge + gather: the 4 per-chunk top-8 sets merge via one max8 over
    [17,32]; an is_equal*rank repair picks the smallest flat rank 4p+c
    among values tied at the global max (column order != flat order;
    duplicated fp32 maxes happen, so this matters); ONE
    indirect_dma_start gathers the 17 winning 2 KB chunks. Chunk size
    == image width, so yy = rank and xx = the within-chunk max_index
    (tie-safe: chunk order == flat order), which is the only DVE op
    after the gather lands. A confidence-gated copy_predicated
    assembles the output; one DMA writes all 17 rows.
  The PE identity and the row-base iota are host-provided inputs.
"""

import sys
from contextlib import ExitStack

for _p in ("/opt/trn_rl_repo", "/opt/pypackages"):
    if _p not in sys.path:
        sys.path.append(_p)

import numpy as np

import concourse.bacc as bacc
import concourse.bass as bass
import concourse.tile as tile
from concourse import mybir
from concourse.bass_utils import run_bass_kernel_spmd

K = 17          # keypoints per core
NK = 133        # total keypoints
ROW = 262144    # 512*512
P = 128         # SBUF partitions
F = ROW // P    # 2048 free elems per partition
C = 4           # chunks per partition row
S = F // C      # 512 elems per chunk
W = 512
N_CORES = 8

f32 = mybir.dt.float32
i32 = mybir.dt.int32
u32 = mybir.dt.uint32
Alu = mybir.AluOpType
Act = mybir.ActivationFunctionType
X = mybir.AxisListType.X

_NC_CACHE = None


def _build():
    nc = bacc.Bacc("TRN2", target_bir_lowering=False, debug=False)
    x_dram = nc.dram_tensor("x", [K, ROW], f32, kind="ExternalInput")
    ident_dram = nc.dram_tensor("ident", [P, P], f32, kind="ExternalInput")
    kiota_dram = nc.dram_tensor("kiota", [K, 1], f32, kind="ExternalInput")
    out_dram = nc.dram_tensor("out", [K, 3], f32, kind="ExternalOutput")

    x_pkf = x_dram.ap().rearrange("k (p f) -> p k f", f=F)      # [128, K, 2048]
    x_rows = x_dram.ap().rearrange("k (r s) -> (k r) s", s=S)   # [K*512, 512]

    with tile.TileContext(nc) as tc, ExitStack() as ctx:
        const_pool = ctx.enter_context(tc.tile_pool(name="const", bufs=1))
        in_pool = ctx.enter_context(tc.tile_pool(name="in", bufs=K))
        small_pool = ctx.enter_context(tc.tile_pool(name="small", bufs=1))
        psum_pool = ctx.enter_context(
            tc.tile_pool(name="psum", bufs=1, space="PSUM"))

        # stream DMAs first so descriptors hit the queues ASAP; one DMA
        # per keypoint for k0..k14, quarters for k15 and k16 so their
        # reduces (and the chunk selects behind them) chase the tail
        # closely.
        KF = K - 2
        tiles_sb = []
        for k in range(K):
            t = in_pool.tile([P, F], f32, tag="xin")
            if k < KF:
                nc.sync.dma_start(t[:], x_pkf[:, k, :])
            tiles_sb.append((k, t))
        for k in (KF, KF + 1):
            t = tiles_sb[k][1]
            for c in range(C):
                nc.sync.dma_start(t[:, c * S:(c + 1) * S],
                                  x_pkf[:, k, c * S:(c + 1) * S])

        ident = const_pool.tile([P, P], f32)
        nc.scalar.dma_start(ident[:], ident_dram.ap())
        kiota = const_pool.tile([K, 1], f32)   # kiota[k] = 512*k + 513
        nc.scalar.dma_start(kiota[:], kiota_dram.ap())

        out_sb = small_pool.tile([K, 3], f32)
        nc.vector.memset(out_sb[:, 0:2], -4.0)
        nc.vector.memset(out_sb[:, 2:3], -1.0)
        # search-value tile for the post-gather max_index: col 0 gets
        # gmax later; the -1e30 floor keeps it "sorted descending" as
        # the firmware expects of an InstMax output
        gm8 = small_pool.tile([K, 8], f32)
        nc.vector.memset(gm8[:], -1e30)

        # per-(partition, chunk) maxes, chunk columns in keypoint-major order
        pmax = small_pool.tile([P, K * C], f32)
        pm3 = pmax[:].rearrange("p (k c) -> p k c", c=C)
        psumT = psum_pool.tile([K, C * P], f32)
        warm = psum_pool.tile([1, P], f32, tag="warm")
        for k, t in tiles_sb[:KF]:
            t3 = t[:].rearrange("p (c s) -> p c s", s=S)        # [P, C, S]
            nc.vector.reduce_max(
                pmax[:, k * C:(k + 1) * C], t3[:, :, :], axis=X)
            if k in (9, 12, 14):
                # keep the PE p-state warm for the select transposes
                nc.tensor.matmul(warm[:], pm3[:, 0:1, 0], ident[:],
                                 is_transpose=True)
        t3_15 = tiles_sb[KF][1][:].rearrange("p (c s) -> p c s", s=S)
        for c in range(C):
            nc.vector.reduce_max(
                pmax[:, KF * C + c:KF * C + c + 1],
                t3_15[:, c:c + 1, :], axis=X)
        # per-chunk select, pipelined behind k16's quarter reduces:
        # psumT[k, c*128+p] = chunkmax(p, c); within chunk c, column j
        # is partition j, so flat rank = 4*j + c decodes in one fused op
        t3_16 = tiles_sb[KF + 1][1][:].rearrange("p (c s) -> p c s", s=S)
        vm_all = small_pool.tile([K, C * 8], f32)   # per-chunk top-8 values
        v5_all = small_pool.tile([K, C * 8], f32)   # 513 - rank per entry
        vi_c = []
        for c in range(C):
            nc.vector.reduce_max(
                pmax[:, (KF + 1) * C + c:(KF + 1) * C + c + 1],
                t3_16[:, c:c + 1, :], axis=X)
            nc.tensor.matmul(psumT[:, c * P:(c + 1) * P],
                             pm3[:, :, c], ident[:], is_transpose=True)
            nc.vector.max(vm_all[:, c * 8:(c + 1) * 8],
                          psumT[:, c * P:(c + 1) * P])
            vi = small_pool.tile([K, 8], u32, tag=f"vi{c}")
            nc.vector.max_index(vi[:], vm_all[:, c * 8:(c + 1) * 8],
                                psumT[:, c * P:(c + 1) * P])
            vi_c.append(vi)
            nc.vector.tensor_scalar(
                v5_all[:, c * 8:(c + 1) * 8], vi[:], -4.0,
                float(513 - c), Alu.mult, Alu.add)

        # merge the 4 chunk top-8 sets: global max + smallest tied rank
        gmx8 = small_pool.tile([K, 8], f32)
        nc.vector.max(gmx8[:], vm_all[:])
        gmax = gmx8[:, 0:1]
        cand = small_pool.tile([K, C * 8], f32)
        nc.vector.scalar_tensor_tensor(
            cand[:], in0=vm_all[:], scalar=gmax, in1=v5_all[:],
            op0=Alu.is_equal, op1=Alu.mult)
        rc = small_pool.tile([K, 1], f32)          # 513 - min tied rank
        nc.vector.reduce_max(rc[:], cand[:], axis=X)
        # gather row = 512*k + rank = kiota - rc  (kiota = 512k + 513)
        offs_i = small_pool.tile([K, 1], i32)
        nc.vector.scalar_tensor_tensor(
            offs_i[:], in0=rc[:], scalar=-1.0, in1=kiota[:],
            op0=Alu.mult, op1=Alu.add)

        # one gather for all 17 winning chunks
        grow = small_pool.tile([K, S], f32)
        nc.gpsimd.indirect_dma_start(
            out=grow[:], out_offset=None, in_=x_rows,
            in_offset=bass.IndirectOffsetOnAxis(ap=offs_i[:, 0:1], axis=0))

        # off-critical-path decode prep (runs while the gather is in flight)
        cand3 = small_pool.tile([K, 3], f32)
        nc.scalar.activation(cand3[:, 2:3], gmax, Act.Sigmoid)
        # yy = rank (chunk size == W): 4*yy = 4*(513 - rc) = 2052 - 4*rc
        nc.vector.tensor_scalar(cand3[:, 1:2], rc[:], -4.0, 2052.0,
                                Alu.mult, Alu.add)
        valid = small_pool.tile([K, 1], f32)
        nc.vector.tensor_scalar(valid[:], cand3[:, 2:3], 0.8, None, Alu.is_gt)
        vb3 = small_pool.tile([K, 3], i32)
        nc.vector.tensor_scalar(vb3[:], out_sb[:], 0.0, valid[:],
                                Alu.mult, Alu.add)
        nc.vector.copy_predicated(out_sb[:, 1:3], vb3[:, 1:3], cand3[:, 1:3])
        nc.vector.tensor_copy(gm8[:, 0:1], gmax)
        # scalars for the fused x-column decode: out0 = j*4v + (4v - 4)
        # = valid ? 4*j : -4
        fv = small_pool.tile([K, 1], f32)
        nc.vector.tensor_scalar(fv[:], valid[:], 4.0, None, Alu.mult)
        fvm4 = small_pool.tile([K, 1], f32)
        nc.vector.tensor_scalar(fvm4[:], fv[:], -4.0, None, Alu.add)

        # index within the winning chunk == xx; the max value is already
        # known (gmax == max of the gathered row), so only max_index runs
        # after the gather lands
        jidx8 = small_pool.tile([K, 8], u32)
        nc.vector.max_index(jidx8[:], gm8[:], grow[:])
        nc.vector.scalar_tensor_tensor(
            out_sb[:, 0:1], in0=jidx8[:, 0:1], scalar=fv[:], in1=fvm4[:],
            op0=Alu.mult, op1=Alu.add)
        nc.scalar.dma_start(out_dram.ap()[:, :], out_sb[:])

    nc.compile()
    return nc


def _get_nc():
    global _NC_CACHE
    if _NC_CACHE is None:
        _NC_CACHE = _build()
    return _NC_CACHE


def _shard(x: np.ndarray) -> list[dict[str, np.ndarray]]:
    xf = np.ascontiguousarray(np.asarray(x, dtype=np.float32).reshape(NK, ROW))
    ident = np.ascontiguousarray(np.eye(P, dtype=np.float32))
    kiota = np.ascontiguousarray(
        (float(S) * np.arange(K) + 513.0).astype(np.float32)[:, None])
    shards = []
    for c in range(N_CORES):
        lo = c * K
        s = xf[lo:min(lo + K, NK)]
        if s.shape[0] < K:
            s = np.concatenate(
                [s, np.zeros((K - s.shape[0], ROW), np.float32)], axis=0)
        shards.append({"x": np.ascontiguousarray(s),
                       "ident": ident, "kiota": kiota})
    return shards


def _run(x, trace=False, **kw):
    nc = _get_nc()
    res = run_bass_kernel_spmd(nc, _shard(x), core_ids=list(range(N_CORES)),
                               trace=trace, **kw)
    out = np.concatenate([r["out"] for r in res.results], axis=0)[:NK]
    return out.astype(np.float32), res


def kernel(x: np.ndarray) -> np.ndarray:
    out, _ = _run(x, trace=False)
    return out
